# revision 2
# baseline (speedup 1.0000x reference)
"""2-layer GAT on 8 trn2 cores.

Strategy (dst-sharded, per spec hint):
  - nodes sharded 6250/core by dst; edges bucketed by dst shard, grouped into
    128-dst blocks, split low/high src (int16 gather index limit), padded to a
    uniform per-block chunk grid so one SPMD program serves all cores.
  - K1: h1cat = x @ [W1 | W1@Asrc | W1@Adst]  (node-shard linear)
  - host: assemble full bf16 gather table G1 [N,384] (768B rows)
  - K2: per-edge dma_gather of G1[src] + a_dst[dst] rows; ee=exp(lrelu(asrc+adst));
        segment aggregation as one-hot matmul into PSUM (numer|denom), divide,
        +bias1, ELU, transpose, @W2cat -> h2cat shard
  - host: assemble G2 [N,128] bf16
  - K3: same edge phase for layer 2 (1 head), +bias2 -> out shard [6250,64] f32
Softmax max-subtraction is skipped: alpha = exp(e)/sum(exp(e)) is identical
mathematically and e is O(5), safe in f32.
"""

import sys

sys.path.insert(0, "/opt/trn_rl_repo")

from contextlib import ExitStack

import ml_dtypes
import numpy as np

from concourse import bacc, bass, mybir
from concourse.bass_utils import run_bass_kernel_spmd
from concourse.masks import make_identity
from concourse.tile import TileContext

P = 128
N = 50000
NCORE = 8
SHARD = 6250
SHARD_PAD = 6272  # 49*128
NB = 49  # dst blocks per core
IN_C = 128
HEADS = 8
HID = 32
C1 = HEADS * HID  # 256
CAT1 = C1 + 2 * HEADS  # 272
OUT_C = 64
CAT2 = OUT_C + 2  # 66
LO = 32768
NEG_SLOPE = 0.2
EPS = 1e-16
G1C = 384  # bf16 cols -> 768B rows
G2C = 128  # bf16 cols -> 256B rows
ADC = 64  # f32 cols -> 256B rows ; col 0..H-1 = a_dst, col 8 = dstloc
GROUP = 3  # dst blocks per gather call

f32 = mybir.dt.float32
bf16 = mybir.dt.bfloat16
i16 = mybir.dt.int16
i32 = mybir.dt.int32
AF = mybir.ActivationFunctionType
OP = mybir.AluOpType

CORE_IDS = list(range(NCORE))

# col perm: new c-major col j = c*HEADS + h  <-  old h-major col h*HID + c
PERM_CM = (np.arange(C1) % HEADS) * HID + (np.arange(C1) // HEADS)


def _wrap16(a):
    """int16 idx list -> [128, len/16] layout dma_gather expects."""
    assert len(a) % 16 == 0
    w = a.astype(np.int16).reshape(-1, 16).T  # [16, S]
    return np.tile(w, (8, 1)).copy()  # [128, S]


def _prep_graph(edge_index):
    """Bucket/pad edges. Per-block-index chunk counts (max over cores), so
    padding tracks each block's actual degree. Returns per-core int16 index
    arrays + chunk-count tuples LPs/HPs."""
    src = np.asarray(edge_index[0], np.int64)
    dst = np.asarray(edge_index[1], np.int64)
    core = dst // SHARD
    per_core = []
    maxlo = np.ones(NB, np.int64)
    maxhi = np.ones(NB, np.int64)
    for c in range(NCORE):
        m = core == c
        s, d = src[m], dst[m] - c * SHARD
        blk = d // P
        order = np.argsort(blk, kind="stable")
        s, d, blk = s[order], d[order], blk[order]
        lows, highs = [], []
        for b in range(NB):
            bm = blk == b
            sb, db = s[bm], d[bm]
            lm = sb < LO
            lows.append((sb[lm], db[lm]))
            highs.append((sb[~lm] - LO, db[~lm]))
            maxlo[b] = max(maxlo[b], len(lows[-1][0]))
            maxhi[b] = max(maxhi[b], len(highs[-1][0]))
        per_core.append((lows, highs))
    LPs = tuple(int(-(-v // P)) for v in maxlo)
    HPs = tuple(int(-(-v // P)) for v in maxhi)
    OL = np.concatenate([[0], np.cumsum(LPs)])  # chunk offsets per block
    OH = np.concatenate([[0], np.cumsum(HPs)])
    out = []
    for c in range(NCORE):
        lows, highs = per_core[c]
        arrs = {}
        for tag, lists, offs in (("l", lows, OL), ("h", highs, OH)):
            tot = int(offs[-1]) * P
            gi = np.zeros(tot, np.int64)
            ai = np.full(tot, SHARD, np.int64)
            for b, (sb, db) in enumerate(lists):
                o = int(offs[b]) * P
                gi[o : o + len(sb)] = sb
                ai[o : o + len(db)] = db
            arrs["g" + tag] = _wrap16(gi)
            arrs["a" + tag] = _wrap16(ai)
        out.append(arrs)
    return out, LPs, HPs


def _build_k1():
    nc = bacc.Bacc("TRN2", target_bir_lowering=False)
    xT = nc.dram_tensor("xT", [P, SHARD_PAD], f32, kind="ExternalInput")
    Wc = nc.dram_tensor("w1cat", [P, CAT1], f32, kind="ExternalInput")
    out = nc.dram_tensor("h1cat", [SHARD_PAD, CAT1], bf16, kind="ExternalOutput")
    with TileContext(nc) as tc, ExitStack() as ctx:
        sb = ctx.enter_context(tc.tile_pool(name="sb", bufs=2))
        con = ctx.enter_context(tc.tile_pool(name="con", bufs=1))
        ps = ctx.enter_context(tc.tile_pool(name="ps", bufs=4, space="PSUM"))
        w = con.tile([P, CAT1], f32)
        nc.sync.dma_start(out=w[:], in_=Wc[:])
        xt = con.tile([P, SHARD_PAD], f32)
        for q in range(4):
            w4 = SHARD_PAD // 4
            nc.sync.dma_start(
                out=xt[:, q * w4 : (q + 1) * w4], in_=xT[:, q * w4 : (q + 1) * w4]
            )
        for g in range(7):
            ot = sb.tile([P, 7, CAT1], bf16)
            for u in range(7):
                t = g * 7 + u
                pt = ps.tile([P, CAT1], f32)
                nc.tensor.matmul(
                    pt[:], lhsT=xt[:, t * P : (t + 1) * P], rhs=w[:],
                    start=True, stop=True,
                )
                if u % 2 == 0:
                    nc.vector.tensor_copy(out=ot[:, u, :], in_=pt[:])
                else:
                    nc.scalar.activation(out=ot[:, u, :], in_=pt[:], func=AF.Copy)
            nc.sync.dma_start(
                out=out[g * 7 * P : (g + 1) * 7 * P, :].rearrange(
                    "(t p) c -> p t c", p=P
                ),
                in_=ot[:],
            )
    nc.compile()
    return nc


def _build_edge_kernel(layer, LP, HP):
    """layer 1: gather G1 768B rows (h1 block stored c-major), H=8,
    out h2cat [SHARD_PAD, CAT2].
    layer 2: gather G2 256B rows (col 64 = const 1, col 65 = a2src), H=1,
    ee folded into the one-hot (scaled S^T), out [SHARD_PAD, OUT_C] (+bias2)."""
    H = HEADS if layer == 1 else 1
    GC = G1C if layer == 1 else G2C
    CW = C1 if layer == 1 else OUT_C
    AS0 = C1 if layer == 1 else OUT_C + 1  # a_src col in gathered row
    MW = CW + H  # psum payload: msg | ee
    LPs, HPs = LP, HP  # per-block chunk-count tuples
    OL = [0]
    for v in LPs:
        OL.append(OL[-1] + v)
    OH = [0]
    for v in HPs:
        OH.append(OH[-1] + v)
    SL = OL[-1] * P // 16
    SH = OH[-1] * P // 16
    n_grp_ = -(-NB // GROUP)
    NCHMAX = max(
        max(OL[min(g * GROUP + GROUP, NB)] - OL[g * GROUP] for g in range(n_grp_)),
        max(OH[min(g * GROUP + GROUP, NB)] - OH[g * GROUP] for g in range(n_grp_)),
    )

    nc = bacc.Bacc("TRN2", target_bir_lowering=False)
    G = nc.dram_tensor("gtab", [N, GC], bf16, kind="ExternalInput")
    AD = nc.dram_tensor("adtab", [SHARD_PAD, ADC], f32, kind="ExternalInput")
    gl = nc.dram_tensor("gl", [P, SL], i16, kind="ExternalInput")
    al = nc.dram_tensor("al", [P, SL], i16, kind="ExternalInput")
    gh = nc.dram_tensor("gh", [P, SH], i16, kind="ExternalInput")
    ah = nc.dram_tensor("ah", [P, SH], i16, kind="ExternalInput")
    if layer == 1:
        W2c = nc.dram_tensor("w2cat", [P, 2 * CAT2], f32, kind="ExternalInput")
        BIAS = nc.dram_tensor("bias", [P, C1], f32, kind="ExternalInput")
        OUT = nc.dram_tensor("h2cat", [SHARD_PAD, CAT2], f32, kind="ExternalOutput")
    else:
        BIAS = nc.dram_tensor("bias", [P, OUT_C], f32, kind="ExternalInput")
        OUT = nc.dram_tensor("out", [SHARD_PAD, OUT_C], f32, kind="ExternalOutput")

    n_grp = -(-NB // GROUP)

    with TileContext(nc) as tc, ExitStack() as ctx:
        con = ctx.enter_context(tc.tile_pool(name="con", bufs=1))
        gpool = ctx.enter_context(tc.tile_pool(name="g", bufs=2))
        apool = ctx.enter_context(tc.tile_pool(name="a", bufs=2))
        spool = ctx.enter_context(tc.tile_pool(name="s", bufs=2))
        mpool = ctx.enter_context(tc.tile_pool(name="m", bufs=2))
        epool = ctx.enter_context(tc.tile_pool(name="e", bufs=2))
        psagg = ctx.enter_context(tc.tile_pool(name="pa", bufs=4, space="PSUM"))
        pstp = ctx.enter_context(tc.tile_pool(name="pt", bufs=2, space="PSUM"))
        psmm = ctx.enter_context(tc.tile_pool(name="pm", bufs=2, space="PSUM"))

        glt = con.tile([P, SL], i16)
        nc.sync.dma_start(out=glt[:], in_=gl[:])
        alt = con.tile([P, SL], i16)
        nc.sync.dma_start(out=alt[:], in_=al[:])
        ght = con.tile([P, SH], i16)
        nc.sync.dma_start(out=ght[:], in_=gh[:])
        aht = con.tile([P, SH], i16)
        nc.sync.dma_start(out=aht[:], in_=ah[:])
        bias_t = con.tile([P, CW], f32)
        nc.sync.dma_start(out=bias_t[:], in_=BIAS[:])
        # iota_exp[p, d, a] = d  (bf16, one-time)
        iota_i = con.tile([P, P], i32)
        nc.gpsimd.iota(iota_i[:], pattern=[[1, P]], base=0, channel_multiplier=0)
        iota_exp = con.tile([P, P, NCHMAX], bf16)
        nc.vector.tensor_copy(
            out=iota_exp[:],
            in_=iota_i[:][:, :, None].to_broadcast([P, P, NCHMAX]),
        )
        if layer == 1:
            ident = con.tile([P, P], f32)
            make_identity(nc, ident[:])
            w2t = con.tile([P, 2 * CAT2], f32)
            nc.sync.dma_start(out=w2t[:], in_=W2c[:])

        def do_stream(g_idx, a_idx, offs, grp, nblk, tabslice):
            c0 = offs[grp * GROUP]
            nch = offs[grp * GROUP + nblk] - c0
            ne = nch * P
            q0 = c0 * P // 16
            gt = gpool.tile([P, NCHMAX, GC], bf16, tag="gt")
            nc.gpsimd.dma_gather(
                gt[:, :nch, :], tabslice, g_idx[:, q0 : q0 + ne // 16],
                ne, ne, GC, single_packet=False,
            )
            at = apool.tile([P, NCHMAX, ADC], f32, tag="at")
            nc.gpsimd.dma_gather(
                at[:, :nch, :], AD[:], a_idx[:, q0 : q0 + ne // 16],
                ne, ne, ADC, single_packet=False,
            )
            et = epool.tile([P, NCHMAX, H], f32, tag="et")
            nc.vector.tensor_tensor(
                out=et[:, :nch, :], in0=gt[:, :nch, AS0 : AS0 + H],
                in1=at[:, :nch, 0:H], op=OP.add,
            )
            et2 = epool.tile([P, NCHMAX, H], f32, tag="et2")
            nc.vector.tensor_scalar(
                out=et2[:, :nch, :], in0=et[:, :nch, :], scalar1=NEG_SLOPE,
                scalar2=None, op0=OP.mult,
            )
            nc.vector.tensor_tensor(
                out=et2[:, :nch, :], in0=et[:, :nch, :], in1=et2[:, :nch, :],
                op=OP.max,
            )
            eet = epool.tile([P, NCHMAX, H], bf16, tag="eet")
            nc.scalar.activation(out=eet[:, :nch, :], in_=et2[:, :nch, :], func=AF.Exp)
            dl = epool.tile([P, NCHMAX], bf16, tag="dl")
            nc.vector.tensor_copy(out=dl[:, :nch], in_=at[:, :nch, 8])
            # S^T in [e, d, chunk] layout: st2[p, d, a] = (dl[p, a] == d)
            st2 = spool.tile([P, P, NCHMAX], bf16, tag="st")
            nc.vector.tensor_tensor(
                out=st2[:, :, :nch],
                in0=dl[:, :nch][:, None, :].to_broadcast([P, P, nch]),
                in1=iota_exp[:, :, :nch],
                op=OP.is_equal,
            )
            if layer == 2:
                nc.vector.tensor_tensor(
                    out=st2[:, :, :nch], in0=st2[:, :, :nch],
                    in1=eet[:, :nch, 0][:, None, :].to_broadcast([P, P, nch]),
                    op=OP.mult,
                )
                return st2, gt
            mt = mpool.tile([P, NCHMAX, MW], bf16, tag="mt")
            nc.vector.tensor_tensor(
                out=mt[:, :nch, 0:CW].rearrange("p a (c h) -> p a c h", c=HID),
                in0=gt[:, :nch, 0:CW].rearrange("p a (c h) -> p a c h", c=HID),
                in1=eet[:, :nch, :][:, :, None, :].to_broadcast([P, nch, HID, H]),
                op=OP.mult,
            )
            nc.vector.tensor_copy(out=mt[:, :nch, CW:MW], in_=eet[:, :nch, :])
            return st2, mt

        stage = [None]
        for grp in range(n_grp):
            nblk = min(GROUP, NB - grp * GROUP)
            stL, mtL = do_stream(glt, alt, OL, grp, nblk, G[:, :])
            stH, mtH = do_stream(ght, aht, OH, grp, nblk, G[LO:, :])
            for j in range(nblk):
                b = grp * GROUP + j
                pagg = psagg.tile([P, MW], f32)
                lpb, hpb = LPs[b], HPs[b]
                lo0 = OL[b] - OL[grp * GROUP]
                ho0 = OH[b] - OH[grp * GROUP]
                for i in range(lpb):
                    ch = lo0 + i
                    rhs = mtL[:, ch, :] if layer == 1 else mtL[:, ch, 0 : OUT_C + 1]
                    nc.tensor.matmul(
                        pagg[:], lhsT=stL[:, :, ch], rhs=rhs,
                        start=(i == 0), stop=False,
                    )
                for i in range(hpb):
                    ch = ho0 + i
                    rhs = mtH[:, ch, :] if layer == 1 else mtH[:, ch, 0 : OUT_C + 1]
                    nc.tensor.matmul(
                        pagg[:], lhsT=stH[:, :, ch], rhs=rhs,
                        start=False, stop=(i == hpb - 1),
                    )
                dent = epool.tile([P, H], f32, tag="dent")
                nc.vector.tensor_scalar(
                    out=dent[:], in0=pagg[:, CW:MW], scalar1=EPS, scalar2=None,
                    op0=OP.add,
                )
                rec = epool.tile([P, H], f32, tag="rec")
                nc.vector.reciprocal(out=rec[:], in_=dent[:])
                o1 = epool.tile([P, CW], f32, tag="o1")
                if layer == 1:
                    nc.vector.tensor_tensor(
                        out=o1[:].rearrange("p (c h) -> p c h", c=HID),
                        in0=pagg[:, 0:CW].rearrange("p (c h) -> p c h", c=HID),
                        in1=rec[:][:, None, :].to_broadcast([P, HID, H]),
                        op=OP.mult,
                    )
                else:
                    nc.vector.tensor_tensor(
                        out=o1[:], in0=pagg[:, 0:CW],
                        in1=rec[:].to_broadcast([P, CW]), op=OP.mult,
                    )
                nc.vector.tensor_tensor(out=o1[:], in0=o1[:], in1=bias_t[:], op=OP.add)
                if layer == 2:
                    if b % 7 == 0:
                        stage[0] = epool.tile([P, 7, OUT_C], f32, tag="stage", name="stage")
                    nc.vector.tensor_copy(out=stage[0][:, b % 7, :], in_=o1[:])
                    if b % 7 == 6:
                        g7 = b // 7
                        nc.sync.dma_start(
                            out=OUT[g7 * 7 * P : (g7 + 1) * 7 * P, :].rearrange(
                                "(t p) c -> p t c", p=P
                            ),
                            in_=stage[0][:],
                        )
                    continue
                tmin = epool.tile([P, CW], f32, tag="tmin")
                nc.vector.tensor_scalar(
                    out=tmin[:], in0=o1[:], scalar1=0.0, scalar2=None, op0=OP.min
                )
                texp = epool.tile([P, CW], f32, tag="texp")
                nc.scalar.activation(out=texp[:], in_=tmin[:], func=AF.Exp)
                tmax = epool.tile([P, CW], f32, tag="tmax")
                nc.vector.tensor_scalar(
                    out=tmax[:], in0=o1[:], scalar1=0.0, scalar2=None, op0=OP.max
                )
                helu = epool.tile([P, CW], f32, tag="helu")
                nc.vector.scalar_tensor_tensor(
                    out=helu[:], in0=tmax[:], scalar=-1.0, in1=texp[:],
                    op0=OP.add, op1=OP.add,
                )
                p2 = psmm.tile([P, CAT2], f32)
                for k in range(2):
                    tp = pstp.tile([P, P], f32)
                    nc.tensor.transpose(
                        out=tp[:], in_=helu[:, k * P : (k + 1) * P], identity=ident[:]
                    )
                    hT = epool.tile([P, P], f32, tag="hT")
                    nc.vector.tensor_copy(out=hT[:], in_=tp[:])
                    nc.tensor.matmul(
                        p2[:], lhsT=hT[:], rhs=w2t[:, k * CAT2 : (k + 1) * CAT2],
                        start=(k == 0), stop=(k == 1),
                    )
                if b % 7 == 0:
                    stage[0] = epool.tile([P, 7, CAT2], f32, tag="stage", name="stage")
                nc.vector.tensor_copy(out=stage[0][:, b % 7, :], in_=p2[:])
                if b % 7 == 6:
                    g7 = b // 7
                    nc.sync.dma_start(
                        out=OUT[g7 * 7 * P : (g7 + 1) * 7 * P, :].rearrange(
                            "(t p) c -> p t c", p=P
                        ),
                        in_=stage[0][:],
                    )
    nc.compile()
    return nc


_CACHE = {}
TRACE = False
LAST_EXEC_NS = None
LAST_PROFILE = []
CAPTURE = None  # when a list: append (tag, nc, core0 in_map) per launch


def _run(nc, maps, tag):
    global LAST_EXEC_NS
    if CAPTURE is not None:
        CAPTURE.append((tag, nc, {k: np.copy(v) for k, v in maps[0].items()}))
    r = run_bass_kernel_spmd(nc, maps, CORE_IDS, trace=TRACE)
    if TRACE:
        ns = r.exec_time_ns
        LAST_PROFILE.append((tag, ns))
        if ns is not None:
            LAST_EXEC_NS = (LAST_EXEC_NS or 0) + ns
    return r.results


def _programs(LP, HP):
    key = (LP, HP)
    if key not in _CACHE:
        _CACHE[key] = (
            _build_k1(),
            _build_edge_kernel(1, LP, HP),
            _build_edge_kernel(2, LP, HP),
        )
    return _CACHE[key]


def kernel(
    x, edge_index, W1, att_src1, att_dst1, bias1, W2, att_src2, att_dst2, bias2
):
    x = np.asarray(x, np.float32)
    W1 = np.asarray(W1, np.float32)
    W2 = np.asarray(W2, np.float32)
    att_src1 = np.asarray(att_src1, np.float32)
    att_dst1 = np.asarray(att_dst1, np.float32)
    att_src2 = np.asarray(att_src2, np.float32)
    att_dst2 = np.asarray(att_dst2, np.float32)
    bias1 = np.asarray(bias1, np.float32)
    bias2 = np.asarray(bias2, np.float32)

    idx_arrs, LP, HP = _prep_graph(edge_index)
    nc1, nc2, nc3 = _programs(LP, HP)

    # ---- K1: h1cat = x @ [W1 | W1@Asrc | W1@Adst] ----
    A_s = np.zeros((C1, HEADS), np.float32)
    A_d = np.zeros((C1, HEADS), np.float32)
    for h in range(HEADS):
        A_s[h * HID : (h + 1) * HID, h] = att_src1[h]
        A_d[h * HID : (h + 1) * HID, h] = att_dst1[h]
    W1cat = np.concatenate([W1, W1 @ A_s, W1 @ A_d], axis=1)  # [128, 272]
    xT = np.zeros((NCORE, P, SHARD_PAD), np.float32)
    for c in range(NCORE):
        xT[c, :, :SHARD] = x[c * SHARD : (c + 1) * SHARD].T
    maps1 = [{"xT": xT[c], "w1cat": W1cat} for c in range(NCORE)]
    r1 = _run(nc1, maps1, "k1")
    h1cat = np.concatenate([r1[c]["h1cat"][:SHARD] for c in range(NCORE)])  # [N,272] bf16

    # ---- K2 ----
    G1 = np.zeros((N, G1C), ml_dtypes.bfloat16)
    G1[:, :C1] = h1cat[:, :C1][:, PERM_CM].astype(ml_dtypes.bfloat16)
    G1[:, C1:CAT1] = h1cat[:, C1:CAT1].astype(ml_dtypes.bfloat16)
    dstloc = (np.arange(SHARD_PAD) % P).astype(np.float32)
    W2cat = np.concatenate(
        [W2, W2 @ att_src2.T, W2 @ att_dst2.T], axis=1
    )[PERM_CM]  # [256, 66], rows c-major to match o1 layout
    w2c = np.concatenate([W2cat[:P], W2cat[P:]], axis=1).astype(np.float32)  # [128,132]
    b1bc = np.tile(bias1[PERM_CM][None, :], (P, 1)).astype(np.float32)
    maps2 = []
    for c in range(NCORE):
        ad = np.zeros((SHARD_PAD, ADC), np.float32)
        ad[:SHARD, :HEADS] = h1cat[c * SHARD : (c + 1) * SHARD, CAT1 - HEADS : CAT1]
        ad[:, 8] = dstloc
        ad[SHARD, :] = 0.0
        ad[SHARD, 8] = -1.0
        a = idx_arrs[c]
        maps2.append(
            {
                "gtab": G1, "adtab": ad, "gl": a["gl"], "al": a["al"],
                "gh": a["gh"], "ah": a["ah"], "w2cat": w2c, "bias": b1bc,
            }
        )
    r2 = _run(nc2, maps2, "k2")
    h2cat = np.concatenate([r2[c]["h2cat"][:SHARD] for c in range(NCORE)])  # [N,66]

    # ---- K3 ----
    G2 = np.zeros((N, G2C), ml_dtypes.bfloat16)
    G2[:, :OUT_C] = h2cat[:, :OUT_C].astype(ml_dtypes.bfloat16)
    G2[:, OUT_C] = 1.0  # denom column
    G2[:, OUT_C + 1] = h2cat[:, OUT_C].astype(ml_dtypes.bfloat16)  # a2src
    b2bc = np.tile(bias2[None, :], (P, 1)).astype(np.float32)
    maps3 = []
    for c in range(NCORE):
        ad = np.zeros((SHARD_PAD, ADC), np.float32)
        ad[:SHARD, 0] = h2cat[c * SHARD : (c + 1) * SHARD, CAT2 - 1]
        ad[:, 8] = dstloc
        ad[SHARD, 0] = 0.0
        ad[SHARD, 8] = -1.0
        a = idx_arrs[c]
        maps3.append(
            {
                "gtab": G2, "adtab": ad, "gl": a["gl"], "al": a["al"],
                "gh": a["gh"], "ah": a["ah"], "bias": b2bc,
            }
        )
    r3 = _run(nc3, maps3, "k3")
    return np.concatenate([r3[c]["out"][:SHARD] for c in range(NCORE)]).astype(
        np.float32
    )

